# revision 7
# baseline (speedup 1.0000x reference)
"""Trainium2 Bass kernel for nn_AttnAdapter (GQA attention + RoPE + ClearSight
VAF region scaling + causal softmax), tensor-parallel over heads on 8 cores.

Sharding: core c owns q-heads 4c..4c+3 and kv-head c (Wq/Wk/Wv column shards,
Wo column shard of the output dim). hidden_states^T is AllGathered from 1/8
shards at kernel start; oT is AllGathered before o_proj; final output columns
are concatenated on the host.
"""

import numpy as np

import concourse.bass as bass
import concourse.mybir as mybir
import concourse.tile as tile
from concourse import bacc
from concourse.bass import ts
from concourse.bass_utils import run_bass_kernel_spmd

N_CORES = 8
P = 128
S = 2048
H = 4096
HD = 128
HQ = 4              # q heads per core
JW = 512            # qs super-tile width
NJ = S // JW        # 4
NT = S // P         # 16
KH = H // P         # 32 contraction tiles for projections
KSH = KH // N_CORES  # 4 k-tiles per hsT shard
SYS, IMG = 35, 576
B = SYS + IMG       # 611: first query row with VAF scaling
ENH, SUP = 2.0, 0.1
FT = -(-B // P)     # 5: ks-tiles with non-unit VAF factor
SCALING = HD ** -0.5

F32 = mybir.dt.float32
F32R = mybir.dt.float32r
USE_F32R = True
MM_DT = F32R if USE_F32R else F32


def _mm(ap):
    return ap


def _build():
    nc = bacc.Bacc("TRN2", target_bir_lowering=False, debug=False,
                   num_devices=N_CORES)

    hs_shard = nc.dram_tensor("hs_shard", [KSH, P, S], MM_DT, kind="ExternalInput")
    wq = nc.dram_tensor("wq", [H, HQ * HD], MM_DT, kind="ExternalInput")
    wk = nc.dram_tensor("wk", [H, HD], MM_DT, kind="ExternalInput")
    wv = nc.dram_tensor("wv", [H, HD], MM_DT, kind="ExternalInput")
    wo = nc.dram_tensor("wo", [H, JW], MM_DT, kind="ExternalInput")
    cosT = nc.dram_tensor("cosT", [P, S], MM_DT, kind="ExternalInput")
    sinT = nc.dram_tensor("sinT", [P, S], MM_DT, kind="ExternalInput")
    rotT = nc.dram_tensor("rotT", [P, P], MM_DT, kind="ExternalInput")
    triT = nc.dram_tensor("triT", [P, P], MM_DT, kind="ExternalInput")
    fmask = nc.dram_tensor("fmask", [P, FT * P], MM_DT, kind="ExternalInput")
    idn = nc.dram_tensor("idn", [P, P], MM_DT, kind="ExternalInput")
    ones_col = nc.dram_tensor("ones_col", [P, 1], MM_DT, kind="ExternalInput")
    fvecT = nc.dram_tensor("fvecT", [P, FT], F32, kind="ExternalInput")
    outT = nc.dram_tensor("outT", [JW, S], F32, kind="ExternalOutput")

    with tile.TileContext(nc) as tc:
        with (
            tc.tile_pool(name="dram", bufs=1, space="DRAM") as dpool,
            tc.tile_pool(name="consts", bufs=1) as cpool,
        ):
            hs_bounce = dpool.tile([KSH, P, S], MM_DT, name="hs_bounce")
            hsT_full = dpool.tile([KH, P, S], MM_DT, addr_space="Shared",
                                  name="hsT_full")
            oT_local = dpool.tile([HQ * HD, S], MM_DT, name="oT_local")
            oT_full = dpool.tile([N_CORES * HQ * HD, S], MM_DT,
                                 addr_space="Shared", name="oT_full")

            nc.sync.dma_start(hs_bounce[:], hs_shard[:])
            nc.gpsimd.collective_compute(
                "AllGather", mybir.AluOpType.bypass,
                replica_groups=[list(range(N_CORES))],
                ins=[hs_bounce.opt()], outs=[hsT_full.opt()],
            )

            rot_sb = cpool.tile([P, P], MM_DT, name="rot_sb")
            tri_sb = cpool.tile([P, P], MM_DT, name="tri_sb")
            fm_sb = cpool.tile([P, FT * P], MM_DT, name="fm_sb")
            idn_sb = cpool.tile([P, P], MM_DT, name="idn_sb")
            ones_sb = cpool.tile([P, 1], MM_DT, name="ones_sb")
            fv_sb = cpool.tile([P, FT], F32, name="fv_sb")
            nc.sync.dma_start(rot_sb[:], rotT[:])
            nc.sync.dma_start(tri_sb[:], triT[:])
            nc.sync.dma_start(fm_sb[:], fmask[:])
            nc.sync.dma_start(idn_sb[:], idn[:])
            nc.sync.dma_start(ones_sb[:], ones_col[:])
            nc.sync.dma_start(fv_sb[:], fvecT[:])

            with tc.tile_pool(name="qkv", bufs=1) as qkv_pool:
                qT = qkv_pool.tile([P, HQ, S], MM_DT, name="qT")
                kT = qkv_pool.tile([P, S], MM_DT, name="kT")
                v_sb = qkv_pool.tile([P, NT, HD], MM_DT, name="v_sb")
                kTs = qkv_pool.tile([P, FT * P], MM_DT, name="kTs")

                # ---- Phase 1: projections qT/kT/vT = W^T @ hsT, RoPE,
                #      VAF-scaled kTs, v = transpose(vT) ----
                with (
                    tc.tile_pool(name="pjw", bufs=1) as pjw,
                    tc.tile_pool(name="hs_pool", bufs=5) as hs_pool,
                ):
                    cos_sb = pjw.tile([P, S], MM_DT, name="cos_sb")
                    sin_sb = pjw.tile([P, S], MM_DT, name="sin_sb")
                    vT = pjw.tile([P, S], MM_DT, name="vT")
                    wq_sb = pjw.tile([P, KH, HQ * HD], MM_DT, name="wq_sb")
                    wk_sb = pjw.tile([P, KH, HD], MM_DT, name="wk_sb")
                    wv_sb = pjw.tile([P, KH, HD], MM_DT, name="wv_sb")
                    nc.sync.dma_start(cos_sb[:], cosT[:])
                    nc.sync.dma_start(sin_sb[:], sinT[:])
                    nc.sync.dma_start(wq_sb[:],
                                      wq.rearrange("(k p) m -> p k m", p=P))
                    nc.sync.dma_start(wk_sb[:],
                                      wk.rearrange("(k p) m -> p k m", p=P))
                    nc.sync.dma_start(wv_sb[:],
                                      wv.rearrange("(k p) m -> p k m", p=P))

                    with tc.tile_pool(name="pj_psum", bufs=1,
                                      space="PSUM") as pj_psum:
                      for n in range(NJ):
                        ps_q = [pj_psum.tile([P, JW], F32, tag=f"psq{h}",
                                             name=f"psq{h}_{n}")
                                for h in range(HQ)]
                        ps_k = pj_psum.tile([P, JW], F32, tag="psk",
                                            name=f"psk_{n}")
                        ps_v = pj_psum.tile([P, JW], F32, tag="psv",
                                            name=f"psv_{n}")
                        for k in range(KH):
                            hst = hs_pool.tile([P, JW], MM_DT, tag="hs",
                                               name=f"hs_{n}_{k}")
                            nc.sync.dma_start(hst[:], hsT_full[k, :, ts(n, JW)])
                            st, sp = (k == 0), (k == KH - 1)
                            for h in range(HQ):
                                nc.tensor.matmul(ps_q[h][:],
                                                 _mm(wq_sb[:, k, ts(h, HD)]),
                                                 _mm(hst[:]), start=st, stop=sp)
                            nc.tensor.matmul(ps_k[:], _mm(wk_sb[:, k, :]),
                                             _mm(hst[:]), start=st, stop=sp)
                            nc.tensor.matmul(ps_v[:], _mm(wv_sb[:, k, :]),
                                             _mm(hst[:]), start=st, stop=sp)
                        for h in range(HQ):
                            nc.vector.tensor_copy(qT[:, h, ts(n, JW)],
                                                  ps_q[h][:])
                        nc.vector.tensor_copy(kT[:, ts(n, JW)], ps_k[:])
                        nc.vector.tensor_copy(vT[:, ts(n, JW)], ps_v[:])

                    # ---- RoPE: x <- x*cos + (Rot@x)*sin ----
                    with (
                        tc.tile_pool(name="rp_tmp", bufs=4) as rp_tmp,
                        tc.tile_pool(name="rp_psum", bufs=3,
                                     space="PSUM") as rp_psum,
                    ):
                        targets = [qT[:, h, :] for h in range(HQ)] + [kT[:]]
                        for i, tgt in enumerate(targets):
                            for n in range(NJ):
                                rps = rp_psum.tile([P, JW], F32, tag="rp",
                                                   name=f"rp_{i}_{n}")
                                nc.tensor.matmul(rps[:], _mm(rot_sb[:]),
                                                 _mm(tgt[:, ts(n, JW)]),
                                                 start=True, stop=True)
                                tmp = rp_tmp.tile([P, JW], MM_DT, tag="rt",
                                                  name=f"rt_{i}_{n}")
                                nc.vector.tensor_mul(tmp[:], rps[:],
                                                     sin_sb[:, ts(n, JW)])
                                nc.vector.tensor_mul(tgt[:, ts(n, JW)],
                                                     tgt[:, ts(n, JW)],
                                                     cos_sb[:, ts(n, JW)])
                                nc.vector.tensor_add(tgt[:, ts(n, JW)],
                                                     tgt[:, ts(n, JW)], tmp[:])

                    nc.vector.tensor_mul(kTs[:], kT[:, 0:FT * P], fm_sb[:])

                    with tc.tile_pool(name="tr_psum", bufs=3,
                                      space="PSUM") as tr_psum:
                        for t in range(NT):
                            tp = tr_psum.tile([P, P], MM_DT, tag="tr",
                                              name=f"tr_{t}")
                            nc.tensor.transpose(tp[:], vT[:, ts(t, P)],
                                                idn_sb[:])
                            nc.vector.tensor_copy(v_sb[:, t, :], tp[:])

                # ---- Phase 2: attention in transposed layout ----
                with (
                    tc.tile_pool(name="sc_psum", bufs=3, space="PSUM") as sc_psum,
                    tc.tile_pool(name="ot_psum", bufs=2, space="PSUM") as ot_psum,
                    tc.tile_pool(name="dn_psum", bufs=2, space="PSUM") as dn_psum,
                    tc.tile_pool(name="strip", bufs=4) as strip_pool,
                    tc.tile_pool(name="norm", bufs=3) as norm_pool,
                ):
                    for h in range(HQ):
                        for J in range(NJ):
                            qlo, qhi = J * JW, (J + 1) * JW
                            tmax = qhi // P - 1
                            otp = ot_psum.tile([P, JW], F32, tag="ot",
                                               name=f"ot_{h}_{J}")
                            dnp = dn_psum.tile([1, JW], F32, tag="dn",
                                               name=f"dn_{h}_{J}")
                            for t in range(tmax + 1):
                                o = max(0, t * P - qlo)
                                scp = sc_psum.tile([P, JW], F32, tag="sc",
                                                   name=f"sc_{h}_{J}_{t}")
                                q_ap = qT[:, h, :]
                                needs_vaf = (t * P < B) and (qhi > B)
                                split = max(o, B - qlo) if needs_vaf else JW
                                if needs_vaf and split == o:
                                    # entire strip in the VAF region
                                    nc.tensor.matmul(
                                        scp[:, o:JW], _mm(kTs[:, ts(t, P)]),
                                        _mm(q_ap[:, qlo + o:qhi]),
                                        start=True, stop=True)
                                else:
                                    nc.tensor.matmul(
                                        scp[:, o:JW], _mm(kT[:, ts(t, P)]),
                                        _mm(q_ap[:, qlo + o:qhi]),
                                        start=True, stop=True)
                                    if needs_vaf and split < JW:
                                        # straddling strip: scale the qs >= B
                                        # columns by the per-ks VAF factor
                                        nc.vector.tensor_scalar_mul(
                                            scp[:, split:JW], scp[:, split:JW],
                                            fv_sb[:, t:t + 1])
                                strip = strip_pool.tile([P, JW], MM_DT, tag="st",
                                                        name=f"st_{h}_{J}_{t}")
                                nc.scalar.activation(
                                    strip[:, o:JW], scp[:, o:JW],
                                    mybir.ActivationFunctionType.Exp)
                                if t * P >= qlo:  # diagonal block
                                    nc.vector.tensor_mul(strip[:, o:o + P],
                                                         strip[:, o:o + P],
                                                         tri_sb[:])
                                st, sp = (t == 0), (t == tmax)
                                nc.tensor.matmul(otp[:, o:JW],
                                                 _mm(v_sb[:, t, :]),
                                                 _mm(strip[:, o:JW]),
                                                 start=st, stop=sp)
                                nc.tensor.matmul(dnp[:, o:JW], _mm(ones_sb[:]),
                                                 _mm(strip[:, o:JW]),
                                                 start=st, stop=sp)
                            recip = norm_pool.tile([1, JW], F32, tag="rc",
                                                   name=f"rc_{h}_{J}")
                            nc.vector.reciprocal(recip[:], dnp[:])
                            bc = norm_pool.tile([P, JW], F32, tag="bc",
                                                name=f"bc_{h}_{J}")
                            nc.gpsimd.partition_broadcast(bc[:], recip[:])
                            ot_sb = norm_pool.tile([P, JW], MM_DT, tag="ots",
                                                   name=f"ots_{h}_{J}")
                            nc.vector.tensor_mul(ot_sb[:], otp[:], bc[:])
                            nc.sync.dma_start(oT_local[ts(h, P), ts(J, JW)],
                                              ot_sb[:])

            # ---- Phase 3: AllGather oT, column-sharded o_proj ----
            nc.gpsimd.collective_compute(
                "AllGather", mybir.AluOpType.bypass,
                replica_groups=[list(range(N_CORES))],
                ins=[oT_local.opt()], outs=[oT_full.opt()],
            )

            KB = 8  # k-tiles per staged accumulation block
            with (
                tc.tile_pool(name="oproj", bufs=1) as opj,
                tc.tile_pool(name="op_pool", bufs=KB + 2) as op_pool,
                tc.tile_pool(name="op_psum", bufs=4, space="PSUM") as op_psum,
            ):
                wo_sb = opj.tile([P, KH, JW], MM_DT, name="wo_sb")
                fin_sb = opj.tile([P, HQ, S], F32, name="fin_sb")
                nc.sync.dma_start(wo_sb[:], wo.rearrange("(k p) m -> p k m", p=P))
                for kb in range(KH // KB):
                    strips = []
                    for j in range(KB):
                        k = kb * KB + j
                        s_t = op_pool.tile([P, S], MM_DT, tag="os", name=f"os_{k}")
                        nc.sync.dma_start(s_t[:], oT_full[ts(k, P), :])
                        strips.append(s_t)
                    for hc in range(HQ):
                        for n in range(NJ):
                            pp = op_psum.tile([P, JW], F32, tag="op",
                                              name=f"op_{kb}_{hc}_{n}")
                            for j in range(KB):
                                k = kb * KB + j
                                nc.tensor.matmul(
                                    pp[:], _mm(wo_sb[:, k, ts(hc, P)]),
                                    _mm(strips[j][:, ts(n, JW)]),
                                    start=(j == 0), stop=(j == KB - 1))
                            if kb == 0:
                                nc.vector.tensor_copy(fin_sb[:, hc, ts(n, JW)],
                                                      pp[:])
                            else:
                                nc.vector.tensor_add(fin_sb[:, hc, ts(n, JW)],
                                                     fin_sb[:, hc, ts(n, JW)],
                                                     pp[:])
                nc.sync.dma_start(outT.rearrange("(hc p) s -> p hc s", p=P),
                                  fin_sb[:])

    nc.compile()
    return nc


_NC_CACHE = None


def _get_nc():
    global _NC_CACHE
    if _NC_CACHE is None:
        _NC_CACHE = _build()
    return _NC_CACHE


def _host_inputs(hidden_states, cos, sin, Wq, Wk, Wv, Wo):
    hs2d = np.asarray(hidden_states, dtype=np.float32).reshape(S, H)
    hsT = np.ascontiguousarray(hs2d.T)                      # [H, S]
    cosT_np = np.ascontiguousarray(np.asarray(cos, np.float32).reshape(S, HD).T)
    sinT_np = np.ascontiguousarray(np.asarray(sin, np.float32).reshape(S, HD).T)

    rot = np.zeros((HD, HD), np.float32)
    for i in range(HD // 2):
        rot[i, i + HD // 2] = -1.0
        rot[i + HD // 2, i] = 1.0
    rotT_np = np.ascontiguousarray(rot.T)

    triT_np = np.triu(np.ones((P, P), np.float32))
    f = np.ones(FT * P, np.float32)
    f[:SYS] = SUP
    f[SYS:B] = ENH
    fmask_np = np.ascontiguousarray(np.broadcast_to(f, (P, FT * P)))
    idn_np = np.eye(P, dtype=np.float32)
    ones_np = np.ones((P, 1), np.float32)
    fvecT_np = np.ascontiguousarray(f.reshape(FT, P).T)

    Wq = np.asarray(Wq, np.float32) * np.float32(SCALING)
    Wk = np.asarray(Wk, np.float32)
    Wv = np.asarray(Wv, np.float32)
    Wo = np.asarray(Wo, np.float32)

    in_maps = []
    for c in range(N_CORES):
        in_maps.append({
            "hs_shard": np.ascontiguousarray(
                hsT[c * (H // N_CORES):(c + 1) * (H // N_CORES)]
            ).reshape(KSH, P, S),
            "wq": np.ascontiguousarray(Wq[:, c * HQ * HD:(c + 1) * HQ * HD]),
            "wk": np.ascontiguousarray(Wk[:, c * HD:(c + 1) * HD]),
            "wv": np.ascontiguousarray(Wv[:, c * HD:(c + 1) * HD]),
            "wo": np.ascontiguousarray(Wo[:, c * JW:(c + 1) * JW]),
            "cosT": cosT_np, "sinT": sinT_np, "rotT": rotT_np,
            "triT": triT_np, "fmask": fmask_np, "idn": idn_np,
            "ones_col": ones_np, "fvecT": fvecT_np,
        })
    return in_maps


def kernel(hidden_states, cos, sin, Wq, Wk, Wv, Wo):
    nc = _get_nc()
    in_maps = _host_inputs(hidden_states, cos, sin, Wq, Wk, Wv, Wo)
    res = run_bass_kernel_spmd(nc, in_maps, core_ids=list(range(N_CORES)))
    out = np.empty((S, H), np.float32)
    for c in range(N_CORES):
        out[:, c * JW:(c + 1) * JW] = res.results[c]["outT"].T
    return out.reshape(1, S, H)
